# revision 61
# baseline (speedup 1.0000x reference)
"""Causal multi-head attention (B=2, L=2048, D=1024, H=16) on 8 trn2 cores.

Sharding: DP on batch (2) x TP on heads (4 groups of 4 heads) = 8 cores.

v2: single-pipeline structure.  The qkv projections, attention
(scores/exp/PV), and out-projection all live in ONE interleaved program:
projection and out-projection matmuls are injected as filler tasks
between attention k-blocks so the PE stays dense while ACT churns exp.
All inputs are host-re-laid-out so each tensor loads with 1-4 big
contiguous DMA descriptors (descriptor issue costs ~0.6us each on the
sync queue).  y is written bf16, one descriptor per q-tile.

Per-core compute for its (batch b, head-group g):
  - qT/kT = wqk_g^T @ x_b^T            [512, L]   (head dims on partitions)
  - V     = x_b @ wv_g (+ ones cols)   [L, 4*65]  (per-head ones column so
            the PV matmul also produces softmax denominators)
  - S^T   = K Q^T per (k-block, q-tile), causal-trimmed, both heads of a
            pair row-packed into one concurrent PE pass; exp on ACT;
            multiplicative triangular mask (GpSimd) on diagonal blocks
  - out^T = V_ext^T @ E^T accumulated over k-blocks -> PSUM.  The V
            padding columns are ALL ONES, so the same matmul replicates
            the softmax denominator r across the unused partition half of
            each head's span: the partition-broadcast of r is free.
  - 1/r = exp(-ln r) on ACT (combined natural_log_exp table set) after
            DVE copies shift the r replicas onto the V partitions
            (PSUM-input copies may change base partition)
  - y_part = attn @ w_out[rows of g]   [L, 1024]  (row-parallel out-proj)
Host gathers: y_b = sum_g y_part + (b_qkv_v @ w_out + b_out).

The PV pipeline is GLOBAL: PV matmuls trail the scores/exp front stream
by SKEW blocks across sub-phase boundaries, so boundaries never drain
the PE.  A PE warm-up (junk matmuls during the DMA fill) releases the
HAM clock gate before real work, and a dummy Exp preloads the ACT
tables.  The final q-tile's normfinish/out-proj chain is chunked
per l-tile and reads PSUM directly to shorten the tail.

PSUM plan (8 banks): pss 2x[128,1024]f32 (scores, 4) + pso 1x[128,1024]
(PV accum, 2) + pp 2x[128,512] (shared qk-proj / V-proj / out-proj, 2).
"""

import sys
from contextlib import ExitStack

if "/opt/trn_rl_repo" not in sys.path:
    sys.path.insert(0, "/opt/trn_rl_repo")

import ml_dtypes
import numpy as np

import concourse.bass as bass
import concourse.mybir as mybir
import concourse.tile as tile
from concourse import bacc
from concourse.bass import ts
from concourse.bass_utils import run_bass_kernel_spmd

F32 = mybir.dt.float32
BF16 = mybir.dt.bfloat16
AF = mybir.ActivationFunctionType
OP = mybir.AluOpType

B, D, H = 2, 1024, 16
HD = 64           # head dim
NH = 4            # heads per core
GD = NH * HD      # 256 head dims per core
P = 128
QTW = 512         # q-tile width
VSTR = 193        # per-pair stride in the v tile: [V0(64)|1] + [z32|1|z31|V1(64)]
VW = 2 * VSTR     # v tile width (2 pairs)


class _combined_exp_ln_tables:
    """Make the ACT table-load pass pick `natural_log_exp_and_others` for
    both Exp and Ln so the 1/r = exp(-ln r) path needs no table reloads."""

    def __enter__(self):
        self._orig = bacc.get_activation_tables
        combined = {AF.Exp, AF.Ln}

        def patched(arch):
            tabs = self._orig(arch)
            out = {}
            for name, funcs in tabs.items():
                if name != "natural_log_exp_and_others":
                    funcs = funcs - combined
                out[name] = funcs
            return out

        bacc.get_activation_tables = patched
        return self

    def __exit__(self, *exc):
        bacc.get_activation_tables = self._orig


def vext(vt, pair, hl):
    """lhsT slice of the extended-V tile for (pair, local head hl).
    Both slices are 128 cols wide (FWL-eligible)."""
    base = VSTR * pair
    if hl == 0:
        return vt[:, base : base + 128]         # V at 0-63, r at 64, pad
    return vt[:, base + 65 : base + VSTR]       # ones@32, V at 64-127


def build_nc(L=2048):
    """Build the per-core Bass program. Same program for all 8 cores (SPMD)."""
    DK = D // P       # 8 contraction chunks
    LT = L // P       # 16 l-tiles
    QT = L // QTW     # 4 q-tiles
    QB = QTW // P     # 4 k-blocks per q-tile

    nc = bacc.Bacc("TRN2", target_bir_lowering=False, debug=False, num_devices=8)

    # host-re-laid-out inputs (see shard_inputs)
    xt_d = nc.dram_tensor("xt", [P, QT * DK * QTW], BF16, kind="ExternalInput").ap()
    wqk_d = nc.dram_tensor("wqk", [P, 4 * DK * P], BF16, kind="ExternalInput").ap()
    wv_d = nc.dram_tensor("wv", [P, DK * GD], BF16, kind="ExternalInput").ap()
    wo_d = nc.dram_tensor("wo", [P, 2 * D], BF16, kind="ExternalInput").ap()
    bqk_d = nc.dram_tensor("bqk", [P, 4], F32, kind="ExternalInput").ap()
    mask_d = nc.dram_tensor("mask", [P, P], BF16, kind="ExternalInput").ap()
    vpat_d = nc.dram_tensor("vpat", [P, 65], BF16, kind="ExternalInput").ap()
    y_d = nc.dram_tensor("y", [L, D], BF16, kind="ExternalOutput").ap()

    with tile.TileContext(nc) as tc, ExitStack() as stk:
        # ---------- pools (few pools: each costs entry/exit barriers) ----
        persist = stk.enter_context(tc.tile_pool(name="persist", bufs=1))
        work = stk.enter_context(tc.tile_pool(name="work", bufs=1))
        psum = stk.enter_context(tc.tile_pool(name="psum", bufs=1, space="PSUM"))

        # ---------- SBUF tiles ----------
        xt_sb = persist.tile([P, QT * DK * QTW], BF16, tag="xt", name="xt_sb")
        wqk_sb = persist.tile([P, 4 * DK * P], BF16, tag="wqk", name="wqk_sb")
        wv_sb = persist.tile([P, DK * GD], BF16, tag="wv", name="wv_sb")
        wo_sb = persist.tile([P, 2 * D], BF16, tag="wo", name="wo_sb")
        bqk_sb = persist.tile([P, 4], F32, tag="bqk", name="bqk_sb")
        mask_sb = persist.tile([P, P], BF16, tag="mask", name="mask_sb")
        vpat_sb = persist.tile([P, 65], BF16, tag="vpat", name="vpat_sb")
        scr_sb = persist.tile([P, QTW], BF16, tag="scr", name="scr_sb")
        qk_sb = [persist.tile([P, L], BF16, tag=f"qk{m}", name=f"qk_sb{m}")
                 for m in range(4)]
        v_sb = [persist.tile([P, VW], BF16, tag=f"v{t}", name=f"v_sb{t}")
                for t in range(LT)]
        attn_sb = [persist.tile([P, L], BF16, tag=f"attn{p}", name=f"attn_sb{p}")
                   for p in range(2)]

        # ---------- PE warm-up during the DMA fill ----------
        # ~20 junk matmuls on a zeroed scratch tile keep the PE busy while
        # inputs stream in, so the HAM clock-gate releases (1.2 -> 2.4 GHz)
        # before the first real matmul instead of ~3.4us after it.
        nc.gpsimd.memset(scr_sb[:], 0.0)
        # tiny dummy Exp: pulls the ~2.7us ACT table load into the fill
        # window so the first real exp doesn't pay it
        nc.scalar.activation(scr_sb[0:1, 0:1], scr_sb[0:1, 0:1], AF.Exp)
        warm_ps = psum.tile([P, QTW], F32, tag="pp", name="warm_ps", bufs=2)
        for _ in range(12):
            nc.tensor.matmul(warm_ps[:], scr_sb[:, 0:P], scr_sb[:],
                             start=True, stop=True)

        # ---------- input DMAs, arrival-priority order ----------
        # qk-proj (n=0, m=0/2) and V lt0-3 unblock first; everything is a
        # big contiguous descriptor.
        def wqk_chunk(m):
            nc.sync.dma_start(wqk_sb[:, ts(m, DK * P)], wqk_d[:, ts(m, DK * P)])

        # first weight/x chunks split in halves so the first proj matmuls
        # (contraction chunks 0-3) start as soon as half has landed
        # biggest critical transfer first: the first matmuls need BOTH
        # xq0a and wqk0a, so starting the larger one a descriptor-issue
        # slot (~0.6us) earlier completes the pair earlier
        HQ = DK * QTW // 2
        nc.sync.dma_start(xt_sb[:, 0:HQ], xt_d[:, 0:HQ])
        HM = DK * P // 2
        nc.sync.dma_start(wqk_sb[:, 0:HM], wqk_d[:, 0:HM])
        nc.sync.dma_start(bqk_sb[:], bqk_d)
        nc.sync.dma_start(wqk_sb[:, HM : 2 * HM], wqk_d[:, HM : 2 * HM])
        nc.sync.dma_start(xt_sb[:, HQ : 2 * HQ], xt_d[:, HQ : 2 * HQ])
        wqk_chunk(2)
        nc.sync.dma_start(wv_sb[:], wv_d)
        nc.sync.dma_start(vpat_sb[:], vpat_d)
        nc.sync.dma_start(mask_sb[:], mask_d)
        wqk_chunk(1)
        wqk_chunk(3)
        for q in range(1, QT):
            nc.sync.dma_start(xt_sb[:, ts(q, DK * QTW)], xt_d[:, ts(q, DK * QTW)])
        nc.sync.dma_start(wo_sb[:], wo_d)

        # ---------- building blocks ----------
        def xsl_proj(n, k):
            """x^T chunk k, L-cols [n*512, (n+1)*512) (quarter-major layout)."""
            return xt_sb[:, n * DK * QTW + k * QTW : n * DK * QTW + (k + 1) * QTW]

        def xsl_v(lt, k):
            q, off = divmod(lt, QB)
            o = q * DK * QTW + k * QTW + off * P
            return xt_sb[:, o : o + P]

        def proj_qk(n, m):
            """qk_sb[m][:, n*512:(n+1)*512] = wqk_m^T @ x^T + bias."""
            def task():
                ps = psum.tile([P, QTW], F32, tag="pp", name="pp_t", bufs=2)
                for k in range(DK):
                    nc.tensor.matmul(
                        ps[:],
                        wqk_sb[:, (m * DK + k) * P : (m * DK + k + 1) * P],
                        xsl_proj(n, k),
                        start=(k == 0), stop=(k == DK - 1))
                nc.vector.tensor_scalar(
                    out=qk_sb[m][:, ts(n, QTW)], in0=ps[:],
                    scalar1=bqk_sb[:, m : m + 1], scalar2=None, op0=OP.add)
            return task

        def proj_v(lt):
            """v_sb[lt] <- x l-tile @ wv, packed into [V|ones] slots."""
            def task():
                vv = v_sb[lt][:, 0:VW].rearrange("p (a c) -> p a c", a=2, c=VSTR)
                # ones filler at cols 64:129 per pair (DVE: GpSimd must stay
                # free for the causal-mask multiplies it gates)
                for a in range(2):
                    nc.vector.tensor_copy(vv[:, a, 64:129], vpat_sb[:])
                ps = psum.tile([P, QTW], F32, tag="pp", name="pp_t", bufs=2)
                for k in range(DK):
                    nc.tensor.matmul(
                        ps[:, 0:GD],
                        xsl_v(lt, k),
                        wv_sb[:, ts(k, GD)],
                        start=(k == 0), stop=(k == DK - 1))
                pv = ps[:, 0:GD].rearrange("p (a c) -> p a c", a=2, c=2 * HD)
                nc.vector.tensor_copy(vv[:, :, 0:64], pv[:, :, 0:64])      # heads 0,2
                nc.vector.tensor_copy(vv[:, :, 129:193], pv[:, :, 64:128])  # heads 1,3
            return task

        ytiles = {}

        def outproj(lt, act_evict=False):
            """y rows [lt*128,(lt+1)*128) = attn @ w_out rows; bf16 stage,
            one y DMA per q-tile."""
            def task():
                qt, a = divmod(lt, QB)
                if a == 0:
                    ytiles[qt] = work.tile([P, QB * D], BF16, tag="y",
                                           name="y_t", bufs=2)
                yt = ytiles[qt]
                for nh in range(2):
                    ps = psum.tile([P, QTW], F32, tag="pp", name="pp_t", bufs=2)
                    for c in range(2):
                        nc.tensor.matmul(
                            ps[:],
                            attn_sb[c][:, ts(lt, P)],
                            wo_sb[:, c * D + nh * QTW : c * D + (nh + 1) * QTW],
                            start=(c == 0), stop=(c == 1))
                    ysl = yt[:, a * D + nh * QTW : a * D + (nh + 1) * QTW]
                    if qt == QT - 1:
                        # final q-tile: evictions alternate ACT/DVE (both
                        # read PSUM) to halve the serial eviction chain,
                        # and per-half-l-tile DMA so the drain overlaps
                        # the remaining out-proj matmuls
                        if nh == 0:
                            nc.vector.tensor_copy(ysl, ps[:])
                        else:
                            nc.scalar.copy(ysl, ps[:])
                        nc.sync.dma_start(
                            y_d[lt * P : (lt + 1) * P,
                                nh * QTW : (nh + 1) * QTW], ysl)
                    elif act_evict:
                        # tail qt2 fillers evict via ACT so the final
                        # normfinish multiplies stay at the DVE queue head
                        nc.scalar.copy(ysl, ps[:])
                    else:
                        nc.vector.tensor_copy(ysl, ps[:])
                if qt == QT - 1:
                    pass
                elif a == QB - 1:
                    dst = y_d[qt * QTW : (qt + 1) * QTW, :].rearrange(
                        "(a p) d -> p a d", p=P)
                    src = yt[:].rearrange("p (a d) -> p a d", a=QB)
                    nc.sync.dma_start(dst, src)
            return task

        SKEW = 3
        pend = []           # global PV pipeline: crosses sub-phase bounds

        def sub_phase(pair, qt, tasks=(), nf=None):
            """Scores+exp front stream for one (pair, q-tile); PV matmuls
            trail the GLOBAL front stream by SKEW blocks so sub-phase
            boundaries never drain the PE pipeline.  `tasks` are PE filler
            closures (proj / out-proj); `nf` is the previous sub-phase's
            normfinish, fired at j==SKEW (after its staging popped)."""
            q_t = qk_sb[pair]
            k_t = qk_sb[2 + pair]
            out_ps = psum.tile([P, 2 * QTW], F32, tag="pso", name="ps_o", bufs=1)
            nblk = QB * qt + QB     # k-blocks for this q-tile
            last = pair == 1 and qt == QT - 1
            sc_box = []

            def front(j):
                """Row-packed scores + exp (+ masks) for k-block j.
                Returns a closure emitting the two PV matmuls."""
                sp = psum.tile([P, 2 * QTW], F32, tag="pss", name="ps_s", bufs=2)
                e_t = work.tile([P, 2 * QTW], BF16, tag="e", name="e_t", bufs=4)
                diag = j >= QB * qt
                da = (j - QB * qt) * P if diag else 0
                for hl in range(2):
                    hb = 64 * hl
                    nc.tensor.matmul(
                        sp[:, hl * QTW + da : (hl + 1) * QTW],
                        k_t[hb : hb + 64, ts(j, P)],
                        q_t[hb : hb + 64,
                            qt * QTW + da : (qt + 1) * QTW],
                        start=True, stop=True)
                if da == 0:
                    nc.scalar.activation(e_t[:], sp[:], AF.Exp, scale=0.125)
                else:
                    # single ACT call over both heads' [da:QTW] spans
                    spv = sp[:].rearrange("p (a c) -> p a c", a=2, c=QTW)
                    ev = e_t[:].rearrange("p (a c) -> p a c", a=2, c=QTW)
                    nc.scalar.activation(ev[:, :, da:QTW], spv[:, :, da:QTW],
                                         AF.Exp, scale=0.125)
                if diag:  # triangular masks on idle GpSimd
                    nc.gpsimd.tensor_tensor(
                        out=e_t[:, da : da + P],
                        in0=e_t[:, da : da + P],
                        in1=mask_sb[:], op=OP.mult)
                    nc.gpsimd.tensor_tensor(
                        out=e_t[:, QTW + da : QTW + da + P],
                        in0=e_t[:, QTW + da : QTW + da + P],
                        in1=mask_sb[:], op=OP.mult)

                def emit_pv(j=j, da=da, e_t=e_t):
                    for hl in range(2):
                        nc.tensor.matmul(
                            out_ps[:, hl * QTW + da : (hl + 1) * QTW],
                            vext(v_sb[j], pair, hl),
                            e_t[:, hl * QTW + da : (hl + 1) * QTW],
                            start=(j == 0), stop=(j == nblk - 1))
                    if j == nblk - 1:
                        staging()
                return emit_pv

            def staging():
                # stage unnormalized out^T to SBUF right after the last
                # PV: frees out_ps (bufs=1).  The all-ones V padding made
                # the PV matmul itself replicate r across the unused
                # partition half of each head's span; the rc copies shift
                # those replicas onto the partitions where the V data
                # lives (PSUM input, so differing base partitions are
                # allowed), giving the r broadcast with no DMA.
                # the last sub-phase's out_ps has no successor: skip
                # staging entirely; normfinish reads PSUM directly
                if last:
                    sc_box.append(out_ps)
                    return
                # ONE full-tile copy (V parts + r replicas together) is
                # cheaper than partial copies and releases out_ps
                # (bufs=1) fastest; the r partition-shift happens inside
                # normfinish's Ln on the idle-ish ACT engine
                sc = work.tile([P, 2 * QTW], F32, tag="sc", name="sc_t", bufs=3)
                nc.vector.tensor_copy(sc[:], out_ps[:])
                sc_box.append(sc)

            tasks = list(tasks)
            stride = max(1, (nblk - 1) // max(len(tasks), 1))
            ti = 0
            for j in range(nblk):
                pend.append(front(j))
                while len(pend) > SKEW:
                    pend.pop(0)()
                if nf is not None and j == SKEW:
                    nf()
                if ti < len(tasks) and j >= 1 and (j - 1) % stride == 0:
                    tasks[ti]()
                    ti += 1
            while ti < len(tasks):
                tasks[ti]()
                ti += 1
            if nf is not None and nblk <= SKEW:
                nf()
            return sc_box

        def normfinish(sc_box, pair, qt):
            """1/r = exp(-ln r) on ACT over the full staged tile (the r
            replicas sit on the opposite partition half of each head's
            span; garbage elsewhere is never read) + DVE normalize into
            the bf16 attn tile with cross-partition operands.  Fired a few
            blocks into the NEXT sub-phase (after staging emitted)."""
            last = pair == 1 and qt == QT - 1

            def mults(sc, rc, c0, cw):
                nc.vector.tensor_tensor(
                    out=attn_sb[pair][0:64, qt * QTW + c0 : qt * QTW + c0 + cw],
                    in0=sc[0:64, c0 : c0 + cw],
                    in1=rc[0:64, c0 : c0 + cw], op=OP.mult)
                nc.vector.tensor_tensor(
                    out=attn_sb[pair][64:P, qt * QTW + c0 : qt * QTW + c0 + cw],
                    in0=sc[64:P, QTW + c0 : QTW + c0 + cw],
                    in1=rc[64:P, c0 : c0 + cw], op=OP.mult)

            def task():
                sc = sc_box[0]
                if not last:
                    # partition-shifted Ln moves the r replicas onto the
                    # V partitions while taking the log (ACT has slack)
                    rc = work.tile([P, QTW], F32, tag="bc", name="rc_t",
                                   bufs=3)
                    nc.scalar.activation(rc[0:64, :], sc[64:P, 0:QTW],
                                         AF.Ln)
                    nc.scalar.activation(rc[64:P, :],
                                         sc[0:64, QTW : 2 * QTW], AF.Ln)
                    nc.scalar.activation(rc[:], rc[:], AF.Exp, scale=-1.0)
                    mults(sc, rc, 0, QTW)
                    return
                # the very last normfinish gates the final out-projections
                # and reads its (unstaged) PSUM accumulator directly:
                # software-pipeline per-l-tile r copies / Ln / Exp /
                # multiplies so out-proj l-tiles unblock one by one
                rc = work.tile([P, QTW], F32, tag="bc", name="rc_t", bufs=3)
                CW = QTW // 2
                for c in range(2):
                    c0 = c * CW
                    # partition-shifted Ln straight from PSUM (ACT is the
                    # only idle PSUM-capable engine at the tail)
                    nc.scalar.activation(rc[0:64, c0 : c0 + CW],
                                         sc[64:P, c0 : c0 + CW], AF.Ln)
                    nc.scalar.activation(rc[64:P, c0 : c0 + CW],
                                         sc[0:64, QTW + c0 : QTW + c0 + CW],
                                         AF.Ln)
                    nc.scalar.activation(rc[:, c0 : c0 + CW],
                                         rc[:, c0 : c0 + CW], AF.Exp,
                                         scale=-1.0)
                    if c >= 1:
                        mults(sc, rc, c0 - CW, CW)
                mults(sc, rc, QTW - CW, CW)
            return task

        # ---------- master schedule ----------
        # pre-phase: minimum deps for attention(0,0)
        proj_qk(0, 0)()
        proj_qk(0, 2)()

        # filler task lists per (pair, qt): the qk/V chunks the NEXT
        # sub-phases need, then out-projections as they become ready.
        sched = {}
        # V lt0-3 ride as (0,0) fillers: their matmuls wait on the later-
        # arriving wv weights, and as pre-phase work they head-of-line
        # blocked the (data-ready) first scores on the PE FIFO
        sched[(0, 0)] = [proj_v(0), proj_v(1), proj_v(2), proj_v(3),
                         proj_qk(0, 1), proj_qk(0, 3)]
        sched[(1, 0)] = [proj_qk(1, 0), proj_qk(1, 2), proj_v(4), proj_v(5)]
        sched[(0, 1)] = [proj_qk(1, 1), proj_qk(1, 3), proj_v(6), proj_v(7)]
        sched[(1, 1)] = [proj_qk(2, 0), proj_qk(2, 2), proj_v(8), proj_v(9)]
        sched[(0, 2)] = [proj_qk(2, 1), proj_qk(2, 3), proj_v(10), proj_v(11),
                         outproj(0), outproj(1)]
        sched[(1, 2)] = [proj_qk(3, 0), proj_qk(3, 2), proj_v(12), proj_v(13),
                         outproj(2), outproj(3)]
        sched[(0, 3)] = [proj_qk(3, 1), proj_qk(3, 3), proj_v(14), proj_v(15),
                         outproj(4), outproj(5)]
        sched[(1, 3)] = [outproj(6), outproj(7), outproj(8), outproj(9)]

        nf_prev = None
        for qt in range(QT):
            for pair in range(2):
                sc_box = sub_phase(pair, qt, sched[(pair, qt)], nf=nf_prev)
                nf_prev = normfinish(sc_box, pair, qt)
        last_box = sc_box

        # tail: drain the PV pipeline, then the last normfinish SPLIT
        # around the qt2 fillers: Ln/Exp first (ACT), then op10/11 whose
        # MMs cover the chain on the PE (emitted before the attn writes,
        # avoiding the conservative tile dependency) with ACT-routed
        # evictions, then the multiplies on an otherwise-empty DVE queue,
        # then the qt3 out-projections.
        while pend:
            pend.pop(0)()
        sc = last_box[0]                # (1,3) PSUM accumulator, unstaged
        rc = work.tile([P, QTW], F32, tag="bc", name="rc_t", bufs=3)
        CW = QTW // 2
        for c in range(2):
            c0 = c * CW
            nc.scalar.activation(rc[0:64, c0 : c0 + CW],
                                 sc[64:P, c0 : c0 + CW], AF.Ln)
            nc.scalar.activation(rc[64:P, c0 : c0 + CW],
                                 sc[0:64, QTW + c0 : QTW + c0 + CW], AF.Ln)
            nc.scalar.activation(rc[:, c0 : c0 + CW], rc[:, c0 : c0 + CW],
                                 AF.Exp, scale=-1.0)
        outproj(10, act_evict=True)()
        outproj(11, act_evict=True)()
        for c in range(2):
            c0 = c * CW
            nc.vector.tensor_tensor(
                out=attn_sb[1][0:64, 3 * QTW + c0 : 3 * QTW + c0 + CW],
                in0=sc[0:64, c0 : c0 + CW],
                in1=rc[0:64, c0 : c0 + CW], op=OP.mult)
            nc.vector.tensor_tensor(
                out=attn_sb[1][64:P, 3 * QTW + c0 : 3 * QTW + c0 + CW],
                in0=sc[64:P, QTW + c0 : QTW + c0 + CW],
                in1=rc[64:P, c0 : c0 + CW], op=OP.mult)
        for lt in range(3 * QB, 4 * QB):
            outproj(lt)()               # qt3 out-proj + final y DMAs

    with _combined_exp_ln_tables():
        nc.compile()
    return nc


def make_mask():
    return (np.arange(P)[:, None] <= np.arange(P)[None, :]).astype(
        ml_dtypes.bfloat16)


def make_vpat():
    # all-ones padding: the PV matmul replicates the softmax denominator r
    # across the unused 64 output partitions of each head's span, giving
    # the partition-broadcast of r for free (no DMA broadcast needed).
    return np.ones((P, 65), ml_dtypes.bfloat16)


def shard_inputs(x, w_qkv, b_qkv, w_out, L=2048):
    """Host-side sharding: core c = (batch c//4, head-group c%4).
    All tensors re-laid-out for big contiguous DMA descriptors and
    direct SBUF slicing:
      xt  [128, (q,4)(k,8)(l,512)]  x^T quarter-major
      wqk [128, (m,4)(k,8)(c,128)]  m: q01,q23,k01,k23
      wv  [128, (k,8)(c,256)]
      wo  [128, (c,2)(d,1024)]
      bqk [128, (m,4)]
    """
    x = np.asarray(x, np.float32)
    w_qkv = np.asarray(w_qkv, np.float32)
    b_qkv = np.asarray(b_qkv, np.float32)
    w_out = np.asarray(w_out, np.float32)
    QT, DK = L // QTW, D // P
    xts = []
    for b in range(B):
        xt = (x[b].reshape(QT, QTW, DK, P).transpose(3, 0, 2, 1)
              .reshape(P, QT * DK * QTW).astype(ml_dtypes.bfloat16))
        xts.append(np.ascontiguousarray(xt))
    mask = make_mask()
    vpat = make_vpat()
    in_maps = []
    for c in range(8):
        b, g = divmod(c, 4)
        qs, ks, vs = GD * g, D + GD * g, 2 * D + GD * g
        wqk_core = np.concatenate(
            [w_qkv[:, qs : qs + GD], w_qkv[:, ks : ks + GD]], axis=1)
        wqk = np.ascontiguousarray(
            wqk_core.reshape(DK, P, 4, P).transpose(1, 2, 0, 3)
            .reshape(P, 4 * DK * P).astype(ml_dtypes.bfloat16))
        wv = np.ascontiguousarray(
            w_qkv[:, vs : vs + GD].reshape(DK, P, GD).transpose(1, 0, 2)
            .reshape(P, DK * GD).astype(ml_dtypes.bfloat16))
        wo = np.ascontiguousarray(
            w_out[GD * g : GD * g + GD, :].reshape(2, P, D).transpose(1, 0, 2)
            .reshape(P, 2 * D).astype(ml_dtypes.bfloat16))
        bqk = np.ascontiguousarray(
            np.concatenate([b_qkv[qs : qs + GD], b_qkv[ks : ks + GD]])
            .reshape(4, P).T.astype(np.float32))
        in_maps.append(
            {"xt": xts[b], "wqk": wqk, "wv": wv, "wo": wo, "bqk": bqk,
             "mask": mask, "vpat": vpat}
        )
    return in_maps


_NC_CACHE = {}


def get_nc(L=2048):
    if L not in _NC_CACHE:
        _NC_CACHE[L] = build_nc(L)
    return _NC_CACHE[L]


def gather(results, b_qkv, w_out, b_out, L=2048):
    fix = (np.asarray(b_qkv, np.float32)[2 * D :] @ np.asarray(w_out, np.float32)
           + np.asarray(b_out, np.float32))
    y = np.zeros((B, L, D), np.float32)
    for c in range(8):
        b = c // 4
        y[b] += np.asarray(results[c]["y"], np.float32)
    y += fix[None, None, :]
    return y


def kernel(x, w_qkv, b_qkv, w_out, b_out):
    L = x.shape[1]
    nc = get_nc(L)
    in_maps = shard_inputs(x, w_qkv, b_qkv, w_out, L=L)
    res = run_bass_kernel_spmd(nc, in_maps, core_ids=list(range(8)))
    return gather(res.results, b_qkv, w_out, b_out, L=L)
